# revision 1
# baseline (speedup 1.0000x reference)
"""Trainium2 Bass kernel for nn_AllToAllMoE (degenerate single-group MoE == dense MLP).

reference:  y = gelu(x @ w1 + b1, exact) @ w2 + b2
  x  (16384, 2048) f32
  w1 (2048, 8192) f32, b1 (8192,) f32
  w2 (8192, 2048) f32, b2 (2048,) f32

Strategy: the all_to_all with a single replica group is an identity permutation,
so the problem is a dense 2-layer MLP. TOKENS are sharded across the 8
NeuronCores (data parallel, no collectives). Everything is computed in the
transposed orientation (feature dim on partitions, tokens on the free dim) so
all DMAs are natural row-major; the host transposes x/y and pre-packs weights.

GEMM1 (x @ w1) uses one level of Strassen per 512-token block:
  tokens 512 -> halves 256, K 2048 -> halves 1024, F 8192 -> halves 4096.
  7 products P_i = Ac_i @ Bc_i accumulate in PSUM (FD=256, two products
  packed per 2KB PSUM bank). A-side combos (of x) are precomputed on the
  host and shipped as `xc`; B-side combos (of w1) are built on-chip by the
  Vector engine from a column-pair slab of the permuted `w1p`; products are
  drained PSUM->SBUF (bf16) by the Scalar engine; the C-quadrant combines
  run on Vector+GpSimd(Pool) in bf16; ScalarE applies exact GELU + b1.
  This cuts GEMM1 TensorE rows by 1/8 (the PE is the bottleneck at ~98%
  occupancy in the classical kernel).

GEMM2 (h @ w2) is classical bf16 (its Strassen variant needs a 7/4-sized
h-combo buffer that does not fit SBUF next to everything else).

Compute dtype: bf16 operands with fp32 PSUM accumulation.
"""

import numpy as np
import ml_dtypes

import concourse.bass as bass
import concourse.mybir as mybir
import concourse.tile as tile
from concourse import bacc
from concourse.bass_utils import run_bass_kernel_spmd

P = 128
N_CORES = 8

TOKENS = 16384
HIDDEN = 2048
FFN = 8192

BF16 = mybir.dt.bfloat16
F32 = mybir.dt.float32

NP_BF16 = np.dtype(ml_dtypes.bfloat16)

# fp8e4 DoubleRow on part of GEMM2 was tried and REVERTED: numerics were
# exactly as predicted (rel err 1.36e-2 with 6/64 k-tiles) but mixing DR
# matmuls into the stream dropped the whole PE clock ~21% (216->260ns per
# FD-512 bf16 matmul), a global net loss of ~250us.

# Strassen products (0-indexed i = P1..P7):
#   Ac: 0:A11+A22 1:A21+A22 2:A11 3:A22 4:A11+A12 5:A21-A11 6:A12-A22
#   Bc: 0:B11+B22 1:B11    2:B12-B22 3:B21-B11 4:B22 5:B11+B12 6:B21+B22
#   C11 = P0+P3-P4+P6 ; C12 = P2+P4 ; C21 = P1+P3 ; C22 = P0-P1+P2+P5
# PE issue order: alias-weight products first (no combo dependency).
PORD = [4, 1, 0, 3, 2, 6, 5]


def build_mlp(T, H, F, TB=512, n_cores=N_CORES,
              slab_bufs=3, wc_bufs=2, w2_bufs=5, y_bufs=3):
    """Per-core fused Strassen-GEMM1 + classical-GEMM2 graph (SPMD)."""
    KH = H // P            # 16 contraction tiles for GEMM1
    KHh = KH // 2          # 8 per k-half
    FT = F // P            # 64 ffn tiles
    FTh = FT // 2          # 32 Strassen column-pair iterations
    HT = H // P            # 16 hidden tiles
    KF = F // P            # 64 contraction tiles for GEMM2
    TH = TB // 2           # 256 token half
    W2G = KF // KH         # 4 w2 k-chunks per n-pair
    n_blocks = T // TB
    assert T % TB == 0 and H % (2 * P) == 0 and F % (2 * P) == 0

    nc = bacc.Bacc("TRN2", target_bir_lowering=False, debug=False,
                   num_devices=n_cores)

    BCOLS = -(-(FT + HT) // P) * P
    xc_d = nc.dram_tensor("xc", (7 * H // 2, n_blocks * TH), BF16,
                          kind="ExternalInput").ap()
    w1p_d = nc.dram_tensor("w1p", (H, F), BF16, kind="ExternalInput").ap()
    w2_d = nc.dram_tensor("w2", (F, H), BF16, kind="ExternalInput").ap()
    bc_d = nc.dram_tensor("bc", (P, BCOLS), F32, kind="ExternalInput").ap()
    out_d = nc.dram_tensor("out", (H, T), F32, kind="ExternalOutput").ap()

    xc_r = xc_d.rearrange("(c p) t -> p c t", p=P)     # [128, 7*KHh, nb*TH]
    w1p_r = w1p_d.rearrange("(k p) f -> p k f", p=P)   # [128, KH, F]
    w2_r = w2_d.rearrange("(k p) h -> p k h", p=P)

    GELU = mybir.ActivationFunctionType.Gelu
    IDENT = mybir.ActivationFunctionType.Identity

    with tile.TileContext(nc) as tc:
        with (
            tc.tile_pool(name="const", bufs=1) as const_pool,
            tc.tile_pool(name="xc", bufs=1) as xc_pool,
            tc.tile_pool(name="slab", bufs=slab_bufs) as slab_pool,
            tc.tile_pool(name="wc", bufs=wc_bufs) as wc_pool,
            tc.tile_pool(name="cp", bufs=2) as cp_pool,
            tc.tile_pool(name="cs", bufs=1) as cs_pool,
            tc.tile_pool(name="ht", bufs=1) as ht_pool,
            tc.tile_pool(name="w2", bufs=w2_bufs) as w2_pool,
            tc.tile_pool(name="y", bufs=y_bufs) as y_pool,
            tc.tile_pool(name="ps", bufs=8, space="PSUM") as ps_pool,
        ):
            bc = const_pool.tile([P, BCOLS], F32)
            b1t = bc[:, 0:FT]
            b2t = bc[:, FT:FT + HT]

            hT = ht_pool.tile([P, FT, TB], BF16)

            def emit_combos(wc, slab):
                B11 = slab[:, 0:KHh, 0:P]
                B12 = slab[:, 0:KHh, P:2 * P]
                B21 = slab[:, KHh:KH, 0:P]
                B22 = slab[:, KHh:KH, P:2 * P]
                nc.vector.tensor_add(wc[:, 0], B11, B22)   # Bc0 (P1)
                nc.vector.tensor_sub(wc[:, 1], B12, B22)   # Bc2 (P3)
                nc.vector.tensor_sub(wc[:, 2], B21, B11)   # Bc3 (P4)
                nc.vector.tensor_add(wc[:, 3], B11, B12)   # Bc5 (P6)
                nc.vector.tensor_add(wc[:, 4], B21, B22)   # Bc6 (P7)

            for t in range(n_blocks):
                ts_ = slice(t * TB, (t + 1) * TB)
                xs_ = slice(t * TH, (t + 1) * TH)
                xc = xc_pool.tile([P, 7 * KHh, TH], BF16, tag="xc")

                def slab_dma(j):
                    s = slab_pool.tile([P, KH, 2 * P], BF16, tag="slab",
                                       name="slab")
                    nc.sync.dma_start(
                        out=s[:], in_=w1p_r[:, :, j * 2 * P:(j + 1) * 2 * P])
                    return s

                def xc_dma(i):
                    nc.sync.dma_start(out=xc[:, i * KHh:(i + 1) * KHh, :],
                                      in_=xc_r[:, i * KHh:(i + 1) * KHh, xs_])

                # block prologue: slabs for j=0,1,2 + combos for j=0,1 + xc.
                if t == 0:
                    # cold start: the first matmul (P5, k=0) needs only
                    # slab row KHh and xc row 4*KHh — land those first,
                    # then stream the rest in consumption order.
                    slab0 = slab_pool.tile([P, KH, 2 * P], BF16, tag="slab",
                                           name="slab")
                    nc.sync.dma_start(out=slab0[:, KHh:KHh + 1, :],
                                      in_=w1p_r[:, KHh:KHh + 1, 0:2 * P])
                    nc.sync.dma_start(out=xc[:, 4 * KHh:4 * KHh + 1, :],
                                      in_=xc_r[:, 4 * KHh:4 * KHh + 1, xs_])
                    nc.sync.dma_start(out=slab0[:, KHh + 1:KH, :],
                                      in_=w1p_r[:, KHh + 1:KH, 0:2 * P])
                    nc.sync.dma_start(out=xc[:, 4 * KHh + 1:5 * KHh, :],
                                      in_=xc_r[:, 4 * KHh + 1:5 * KHh, xs_])
                    nc.sync.dma_start(out=slab0[:, 0:KHh, :],
                                      in_=w1p_r[:, 0:KHh, 0:2 * P])
                    xc_dma(1)
                    xc_dma(0)
                    slabs = [slab0, slab_dma(1)]
                    nc.sync.dma_start(out=bc[:], in_=bc_d[:])
                    for i in [3, 2, 6, 5]:
                        xc_dma(i)
                else:
                    slabs = [slab_dma(0), slab_dma(1)]
                    for i in PORD:
                        xc_dma(i)
                wc0 = wc_pool.tile([P, 5, KHh, P], BF16, tag="wc", name="wc")
                emit_combos(wc0, slabs[0])
                wcs = [wc0]

                # ---- phase A: Strassen GEMM1, gelu -> hT ----
                for j in range(FTh):
                    # software pipeline: fetch slab j+2, combo slab j+1 so
                    # the in-order DVE never blocks the PE at a j boundary.
                    if j + 2 < FTh:
                        slabs.append(slab_dma(j + 2))
                    if j + 1 < FTh:
                        wc_n = wc_pool.tile([P, 5, KHh, P], BF16, tag="wc",
                                            name="wc")
                        emit_combos(wc_n, slabs[1])
                        wcs.append(wc_n)
                    slab, wc = slabs[0], wcs[0]
                    B11 = slab[:, 0:KHh, 0:P]
                    B22 = slab[:, KHh:KH, P:2 * P]
                    lhs = {0: wc[:, 0], 1: B11, 2: wc[:, 1], 3: wc[:, 2],
                           4: B22, 5: wc[:, 3], 6: wc[:, 4]}

                    psA = ps_pool.tile([P, TB], F32, tag="ps")  # P5 | P2
                    psB = ps_pool.tile([P, TB], F32, tag="ps")  # P1 | P4
                    psC = ps_pool.tile([P, TB], F32, tag="ps")  # P3 | P7
                    psD = ps_pool.tile([P, TB], F32, tag="ps")  # P6 | --
                    pslot = {4: (psA, 0), 1: (psA, 1), 0: (psB, 0),
                             3: (psB, 1), 2: (psC, 0), 6: (psC, 1),
                             5: (psD, 0)}
                    cp = cp_pool.tile([P, 7, TH], BF16, tag="cp")
                    for i in PORD:
                        pst, half = pslot[i]
                        dst = pst[:, half * TH:(half + 1) * TH]
                        for k in range(KHh):
                            nc.tensor.matmul(
                                dst, lhsT=lhs[i][:, k, :],
                                rhs=xc[:, i * KHh + k, :],
                                start=(k == 0), stop=(k == KHh - 1))
                        nc.scalar.copy(cp[:, i, :], dst)

                    cs = cs_pool.tile([P, 8, TH], BF16, tag="cs")
                    # DVE: C11 chain + C21 + C12
                    nc.vector.tensor_add(cs[:, 0], cp[:, 0], cp[:, 3])
                    nc.vector.tensor_sub(cs[:, 1], cs[:, 0], cp[:, 4])
                    nc.vector.tensor_add(cs[:, 2], cs[:, 1], cp[:, 6])  # C11
                    nc.vector.tensor_add(cs[:, 3], cp[:, 1], cp[:, 3])  # C21
                    nc.vector.tensor_add(cs[:, 4], cp[:, 2], cp[:, 4])  # C12
                    # Pool: C22 chain
                    nc.gpsimd.tensor_sub(cs[:, 5], cp[:, 0], cp[:, 1])
                    nc.gpsimd.tensor_add(cs[:, 6], cs[:, 5], cp[:, 2])
                    nc.gpsimd.tensor_add(cs[:, 7], cs[:, 6], cp[:, 5])  # C22

                    nc.scalar.activation(hT[:, j, 0:TH], cs[:, 2], GELU,
                                         bias=b1t[:, j:j + 1])
                    nc.scalar.activation(hT[:, j, TH:TB], cs[:, 3], GELU,
                                         bias=b1t[:, j:j + 1])
                    nc.scalar.activation(hT[:, FTh + j, 0:TH], cs[:, 4], GELU,
                                         bias=b1t[:, FTh + j:FTh + j + 1])
                    nc.scalar.activation(hT[:, FTh + j, TH:TB], cs[:, 7], GELU,
                                         bias=b1t[:, FTh + j:FTh + j + 1])
                    slabs.pop(0)
                    wcs.pop(0)

                # ---- phase B: yT = w2^T hT + b2 (classical bf16) ----
                for npair in range(HT // 2):
                    nsl = slice(npair * 2 * P, (npair + 1) * 2 * P)
                    w2cs = []
                    for g in range(W2G):
                        w2c = w2_pool.tile([P, KH, 2 * P], BF16, tag="w2c")
                        nc.sync.dma_start(
                            out=w2c[:], in_=w2_r[:, g * KH:(g + 1) * KH, nsl])
                        w2cs.append(w2c)
                    for sub in range(2):
                        n = npair * 2 + sub
                        ps2 = ps_pool.tile([P, TB], F32, tag="ps")
                        for k2 in range(KF):
                            g, kk = divmod(k2, KH)
                            nc.tensor.matmul(
                                ps2[:],
                                lhsT=w2cs[g][:, kk, sub * P:(sub + 1) * P],
                                rhs=hT[:, k2, :],
                                start=(k2 == 0), stop=(k2 == KF - 1))
                        y = y_pool.tile([P, TB], F32, tag="y")
                        if t == n_blocks - 1 and n == HT - 1:
                            # drain the last tile in halves (kernel tail)
                            for hv in range(2):
                                hsl = slice(hv * TB // 2, (hv + 1) * TB // 2)
                                nc.scalar.activation(
                                    y[:, hsl], ps2[:, hsl], IDENT,
                                    bias=b2t[:, n:n + 1])
                                nc.sync.dma_start(
                                    out=out_d[n * P:(n + 1) * P,
                                              t * TB + hv * TB // 2:
                                              t * TB + (hv + 1) * TB // 2],
                                    in_=y[:, hsl])
                        else:
                            nc.scalar.activation(
                                y[:], ps2[:], IDENT, bias=b2t[:, n:n + 1])
                            nc.sync.dma_start(
                                out=out_d[n * P:(n + 1) * P, ts_], in_=y[:])

    nc.compile()
    return nc


def make_in_maps(x, w1, b1, w2, b2, n_cores=N_CORES):
    """Shard FULL f32 inputs into per-core in_maps (host-side layout prep)."""
    T_core = x.shape[0] // n_cores
    H = x.shape[1]
    F = w1.shape[1]
    FT = F // P
    HT = w2.shape[1] // P
    TB = 512
    TH = TB // 2
    n_blocks = T_core // TB
    Hh = H // 2

    # w1 permuted so that Strassen column-pair (j of F-half1, j of F-half2)
    # is a contiguous 256-col slab: w1p[:, j*256 + s*128 + c] = w1[:, s*F/2 + j*128 + c]
    w1p = np.ascontiguousarray(
        w1.reshape(H, 2, F // 256, 128).transpose(0, 2, 1, 3).reshape(H, F)
    ).astype(NP_BF16)
    w2_b = np.ascontiguousarray(w2.astype(NP_BF16))

    b1t = b1.astype(np.float32).reshape(FT, P).T
    b2t = b2.astype(np.float32).reshape(HT, P).T
    BCOLS = -(-(FT + HT) // P) * P
    bcm = np.zeros((P, BCOLS), dtype=np.float32)
    bcm[:, 0:FT] = b1t
    bcm[:, FT:FT + HT] = b2t

    in_maps = []
    for c in range(n_cores):
        xs = x[c * T_core:(c + 1) * T_core].astype(np.float32)
        xc = np.empty((7 * Hh, n_blocks * TH), dtype=NP_BF16)
        for b in range(n_blocks):
            xb = xs[b * TB:(b + 1) * TB]
            A11 = xb[0:TH, 0:Hh]
            A12 = xb[0:TH, Hh:H]
            A21 = xb[TH:TB, 0:Hh]
            A22 = xb[TH:TB, Hh:H]
            combos = (A11 + A22, A21 + A22, A11, A22,
                      A11 + A12, A21 - A11, A12 - A22)
            csl = slice(b * TH, (b + 1) * TH)
            for i, S in enumerate(combos):
                xc[i * Hh:(i + 1) * Hh, csl] = S.T.astype(NP_BF16)
        in_maps.append({"xc": xc, "w1p": w1p, "w2": w2_b, "bc": bcm})
    return in_maps


_CACHE = {}


def _get_nc():
    if "nc" not in _CACHE:
        _CACHE["nc"] = build_mlp(TOKENS // N_CORES, HIDDEN, FFN, TB=512)
    return _CACHE["nc"]


def run(x, w1, b1, w2, b2, trace=False, **kw):
    nc = _get_nc()
    in_maps = make_in_maps(x, w1, b1, w2, b2)
    res = run_bass_kernel_spmd(nc, in_maps, core_ids=list(range(N_CORES)),
                               trace=trace, **kw)
    y = np.concatenate(
        [np.asarray(res.results[i]["out"]).T for i in range(N_CORES)], axis=0)
    return np.ascontiguousarray(y.astype(np.float32)), res


def kernel(x, w1, b1, w2, b2):
    x = np.asarray(x, dtype=np.float32)
    w1 = np.asarray(w1, dtype=np.float32)
    b1 = np.asarray(b1, dtype=np.float32)
    w2 = np.asarray(w2, dtype=np.float32)
    b2 = np.asarray(b2, dtype=np.float32)
    y, _ = run(x, w1, b1, w2, b2, trace=False)
    return y

